# revision 2
# baseline (speedup 1.0000x reference)
"""Trainium2 Bass kernel for nn_EntitiesIndexingHeadRuleBased (nms_detection).

Flipped layout: entities on partitions (4 chunks of 125), relations x {sub,obj}
merged on the free dim (1000 cols).  bf16 map pipeline; fp32 where precision
matters (B-matmul rows, reciprocal).  Score multiply folded into the PE output
transpose via diag(score) matmuls; output written f32 from PSUM.

Phasing: blocks of 4 images — softmax/packs (exp table set) for the block,
then per image rows+maps (sqrt table set) — 4 ACT table loads total.
"""
import sys
sys.path.insert(0, '/opt/trn_rl_repo')

import numpy as np
import bass_rust
import concourse.bass as bass
import concourse.tile as tile
import concourse.tile as tile_mod
from concourse import mybir
from concourse import bass_utils
from concourse.masks import make_identity
from concourse.tile import TileContext

F32 = mybir.dt.float32
BF16 = mybir.dt.bfloat16
FP16 = mybir.dt.float16
AF = mybir.ActivationFunctionType
OP = mybir.AluOpType

import os
B = 64
NE = 500
NR = 500
NC1 = 151
NCL = 150
P = 125
NCH = 4
N_CORES = 8
N_IMG = B // N_CORES
BLK = int(os.environ.get('K2_BLK', '4'))
EPS_D2 = 5e-4
LAM = 1.0 / 256.0
PACK_BF16 = os.environ.get('K2_PACK_BF16', '0') == '1'
USE_POOL = os.environ.get('K2_POOL', '1') == '1'
USE_INPLACE = os.environ.get('K2_INPLACE', '1') == '1'
PACK_A = None  # set below

# ---------------------------------------------------------------------------
# Workarounds for the container's walrus: it rejects instructions carrying
# more than one sync-wait command ("Too many sync wait commands").
# ---------------------------------------------------------------------------

_MAXW = 1


def _patched_drain_and_barrier(self, tick_clock, wait_clock):
    ScopedClock = tile_mod.ScopedClock
    carrier = self.nc.sync.nop(nofuse=True)
    wait_clock.add_sem_waits(carrier.ins,
                             ScopedClock({None: tick_clock.global_clock}))
    si = carrier.ins.sync_info
    waits = list(si.on_wait) if si is not None else []
    if len(waits) > _MAXW:
        carrier.ins.sync_info = bass_rust.SyncInfo(
            on_wait=waits[:_MAXW], on_update=[])
        for i in range(_MAXW, len(waits), _MAXW):
            nop = self.nc.sync.nop(nofuse=True)
            nop.ins.sync_info = bass_rust.SyncInfo(
                on_wait=waits[i:i + _MAXW], on_update=[])
    self.nc.sync.drain()
    self.nc.all_engine_barrier()
    assert self.sems is not None
    popped = self.nc._tile_sem_poison_stack.pop()
    assert popped is self._sem_poison
    self.nc.clear_and_free_semaphores(list(self.sems.allocated().values()))
    self.nc.all_engine_barrier()


TileContext._drain_and_barrier = _patched_drain_and_barrier


def _split_waits(nc, maxw=_MAXW):
    """Hoist excess sync waits onto same-engine NoOps placed just before the
    offending instruction (engine streams execute in order)."""
    for fn in nc.m.functions:
        for blk in fn.blocks:
            newl = []
            changed = False
            for ins in blk.instructions:
                si = ins.sync_info
                waits = list(si.on_wait) if si is not None else []
                if len(waits) > maxw:
                    changed = True
                    carried, rest = waits[:-maxw], waits[-maxw:]
                    for i in range(0, len(carried), maxw):
                        nop = mybir.InstNoOp(
                            name=f"{ins.name}-sw{i}",
                            sync_info=mybir.SyncInfo(
                                on_wait=carried[i:i + maxw], on_update=[]),
                            bass_nofuse=True,
                            engine=ins.engine,
                        )
                        newl.append(nop)
                    ins.sync_info = mybir.SyncInfo(
                        on_wait=rest, on_update=list(si.on_update))
                newl.append(ins)
            if changed:
                blk.instructions = newl


def _bcast(ap, p):
    """[1,N] DRAM AP -> [p,N] partition-broadcast AP (stride-0 partition)."""
    return bass.AP(tensor=ap.tensor, offset=ap.offset,
                   ap=[[0, p]] + list(ap.ap[1:]))


# ---------------------------------------------------------------------------
# Kernel builder
# ---------------------------------------------------------------------------


def _build(n_img, repeat=1):
    nc = bass.Bass("TRN2", target_bir_lowering=False)

    pb = nc.dram_tensor("pred_boxes", [n_img, NE, 4], F32, kind="ExternalInput")
    pl = nc.dram_tensor("pred_logits", [n_img, NE, NC1], F32, kind="ExternalInput")
    rol = nc.dram_tensor("pred_rel_obj_logits", [n_img, NR, NC1], F32, kind="ExternalInput")
    rsl = nc.dram_tensor("pred_rel_sub_logits", [n_img, NR, NC1], F32, kind="ExternalInput")
    rob = nc.dram_tensor("pred_rel_obj_box", [n_img, NR, 4], F32, kind="ExternalInput")
    rsb = nc.dram_tensor("pred_rel_sub_box", [n_img, NR, 4], F32, kind="ExternalInput")
    rv = nc.dram_tensor("pred_rel_vec", [n_img, NR, 4], F32, kind="ExternalInput")
    tsz = nc.dram_tensor("target_sizes", [n_img, 2], F32, kind="ExternalInput")
    out_s = nc.dram_tensor("out_sub", [n_img, NR, NE], F32, kind="ExternalOutput")
    out_o = nc.dram_tensor("out_obj", [n_img, NR, NE], F32, kind="ExternalOutput")
    dbg = (nc.dram_tensor("dbg", [n_img, NCH, P, 2 * NR], F32,
                          kind="ExternalOutput")
           if os.environ.get('K2_DUMP') else None)

    with tile.TileContext(nc) as tc:
        with (
            tc.tile_pool(name="singles", bufs=1) as singles,
            tc.tile_pool(name="io", bufs=3) as io,
            tc.tile_pool(name="sm", bufs=3) as sm,
            tc.tile_pool(name="col", bufs=4) as col,
            tc.tile_pool(name="packs", bufs=int(os.environ.get('K2_PACKBUFS', '4'))) as packs,
            tc.tile_pool(name="scal", bufs=max(BLK, 2)) as scal,
            tc.tile_pool(name="rows", bufs=int(os.environ.get('K2_ROWBUFS', '1'))) as rows,
            tc.tile_pool(name="mp", bufs=2) as mp,
            tc.tile_pool(name="mph", bufs=int(os.environ.get('K2_HEADBUFS', '2'))) as mph,
            tc.tile_pool(name="ps", bufs=4, space="PSUM") as ps,
            tc.tile_pool(name="pso", bufs=4, space="PSUM") as pso,
            tc.tile_pool(name="dr", bufs=2, space="DRAM") as dr,
        ):
            identf = singles.tile([128, 128], F32, tag="identf")
            make_identity(nc, identf)
            identh = singles.tile([128, 128], FP16, tag="identh")
            nc.vector.tensor_copy(out=identh, in_=identf)

            env = dict(locals())
            env.update(dict(
                io=io, sm=sm, col=col, packs=packs, scal=scal, rows=rows,
                mp=mp, mph=mph, ps=ps, pso=pso, dr=dr,
                pb=pb, pl=pl, rol=rol, rsl=rsl, rob=rob, rsb=rsb, rv=rv,
                tsz=tsz, out_s=out_s, out_o=out_o, dbg=dbg,
                identf=identf, identh=identh, nc=nc))

            for _rep in range(repeat):
                for blk in range(n_img // BLK):
                    imgs = list(range(blk * BLK, (blk + 1) * BLK))
                    st = {}
                    for b in imgs:
                        st[b] = _softmax_packs(env, b)
                    for b in imgs:
                        _rows_scalars(env, b, st[b])
                        _maps(env, b, st[b])
    _split_waits(nc)
    return nc


def _softmax_packs(env, b):
    nc = env["nc"]
    io, sm, col, packs, ps = (env[k] for k in ("io", "sm", "col", "packs", "ps"))
    pl, rsl, rol = env["pl"], env["rsl"], env["rol"]
    identf = env["identf"]
    scal = env["scal"]

    padt = FP16
    PAe = packs.tile([128, NE], padt, tag="PAe", name=f"PAe{b}")
    PBe = packs.tile([24, NE], FP16, tag="PBe")
    PAr = packs.tile([128, 2 * NR], padt, tag="PAr", name=f"PAr{b}")
    PBr = packs.tile([24, 2 * NR], FP16, tag="PBr")
    SCOREc = scal.tile([P, NCH], F32, tag="SCOREc")
    ESQc = scal.tile([P, NCH], F32, tag="ESQc")

    for t, ldram, dA, dB, coff in (
        ("ent", pl, PAe, PBe, 0),
        ("rs", rsl, PAr, PBr, 0),
        ("ro", rol, PAr, PBr, NR),
    ):
        LG = io.tile([P, NCH, NC1], F32, tag="LG")
        nc.sync.dma_start(out=LG, in_=ldram[b].rearrange("(j p) c -> p j c", p=P))
        for j in range(NCH):
            E = sm.tile([P, NC1], BF16, tag="E")
            sumc = col.tile([P, 1], F32, tag="sumc")
            nc.scalar.activation(out=E, in_=LG[:, j, :], func=AF.Exp,
                                 accum_out=sumc)
            r = col.tile([P, 1], F32, tag="r")
            nc.vector.reciprocal(r, sumc)
            SQ = sm.tile([P, NCL], BF16, tag="SQ")
            sqc = col.tile([P, 1], F32, tag="sqc")
            nc.scalar.activation(out=SQ, in_=E[:, :NCL], func=AF.Square,
                                 accum_out=sqc)
            r2 = col.tile([P, 1], F32, tag="r2")
            nc.vector.tensor_scalar(out=r2, in0=r, scalar1=r, scalar2=None,
                                    op0=OP.mult)
            PK = sm.tile([P, 152], F32, tag="PK")
            if t == "ent":
                nc.vector.tensor_scalar(out=PK[:, 0:NCL], in0=E[:, :NCL],
                                        scalar1=r, scalar2=None, op0=OP.mult)
                nc.vector.memset(PK[:, 150:151], 1.0)
                # esq (+eps) as per-partition scalar for the Sqrt bias
                nc.vector.tensor_scalar(out=ESQc[:, j:j + 1], in0=sqc,
                                        scalar1=r2, scalar2=EPS_D2,
                                        op0=OP.mult, op1=OP.add)
                mx = col.tile([P, 1], F32, tag="mx")
                nc.vector.tensor_reduce(out=mx, in_=E[:, :NCL],
                                        axis=mybir.AxisListType.X, op=OP.max)
                nc.vector.tensor_scalar(out=SCOREc[:, j:j + 1], in0=mx,
                                        scalar1=r, scalar2=None, op0=OP.mult)
            else:
                rm2 = col.tile([P, 1], F32, tag="rm2")
                nc.vector.tensor_scalar(out=rm2, in0=r, scalar1=-2.0,
                                        scalar2=None, op0=OP.mult)
                nc.vector.tensor_scalar(out=PK[:, 0:NCL], in0=E[:, :NCL],
                                        scalar1=rm2, scalar2=None, op0=OP.mult)
                # rsq row (pairs with the ones row on the ent side)
                nc.vector.tensor_scalar(out=PK[:, 150:151], in0=sqc,
                                        scalar1=r2, scalar2=None, op0=OP.mult)
            nc.vector.memset(PK[:, 151:152], 0.0)
            PSU = ps.tile([128, 512], F32, tag="PSU")
            nc.tensor.transpose(PSU[:, 0:P], PK[:, 0:128], identf[:P, :P])
            nc.tensor.transpose(PSU[:24, 200:200 + P], PK[:, 128:152],
                                identf[:P, :P])
            nc.scalar.copy(out=dA[:, coff + P * j:coff + P * (j + 1)],
                           in_=PSU[:, 0:P])
            nc.scalar.copy(out=dB[:, coff + P * j:coff + P * (j + 1)],
                           in_=PSU[:24, 200:200 + P])
    return dict(PAe=PAe, PBe=PBe, PAr=PAr, PBr=PBr, SCOREc=SCOREc, ESQc=ESQc)


def _rows_scalars(env, b, st):
    nc = env["nc"]
    io, col, scal, rows, ps, dr = (env[k] for k in
                                   ("io", "col", "scal", "rows", "ps", "dr"))
    pb, rsb, rob, rv, tsz = (env[k] for k in ("pb", "rsb", "rob", "rv", "tsz"))
    identf = env["identf"]

    Wt = col.tile([128, 1], F32, tag="Wt")
    Ht = col.tile([128, 1], F32, tag="Ht")
    nc.sync.dma_start(out=Wt, in_=_bcast(tsz[b, 1:2], 128))
    nc.sync.dma_start(out=Ht, in_=_bcast(tsz[b, 0:1], 128))
    nc.vector.tensor_scalar(out=Wt, in0=Wt, scalar1=LAM, scalar2=None,
                            op0=OP.mult)
    nc.vector.tensor_scalar(out=Ht, in0=Ht, scalar1=LAM, scalar2=None,
                            op0=OP.mult)
    nWt = col.tile([128, 1], F32, tag="nWt")
    nHt = col.tile([128, 1], F32, tag="nHt")
    nc.vector.tensor_scalar(out=nWt, in0=Wt, scalar1=-1.0, scalar2=None,
                            op0=OP.mult)
    nc.vector.tensor_scalar(out=nHt, in0=Ht, scalar1=-1.0, scalar2=None,
                            op0=OP.mult)

    def box_lohi(dram):
        BT = io.tile([P, NCH, 4], F32, tag="BT")
        nc.sync.dma_start(out=BT, in_=dram[b].rearrange("(j p) c -> p j c", p=P))
        LO = col.tile([P, NCH, 2], F32, tag="LO")
        HI = col.tile([P, NCH, 2], F32, tag="HI")
        nc.vector.scalar_tensor_tensor(
            out=LO, in0=BT[:, :, 2:4], scalar=-0.5, in1=BT[:, :, 0:2],
            op0=OP.mult, op1=OP.add)
        nc.vector.scalar_tensor_tensor(
            out=HI, in0=BT[:, :, 2:4], scalar=0.5, in1=BT[:, :, 0:2],
            op0=OP.mult, op1=OP.add)
        return LO, HI

    # --- entity scalars: PG cols 0:-ex0 1:-ey0 2:ex1 3:ey1 4:ew 5:eh
    #                             6:earea 7:-ecx 8:-ecy
    LO, HI = box_lohi(pb)
    PG = scal.tile([P, NCH, 9], F32, tag="PG")
    nc.vector.tensor_scalar(out=PG[:, :, 0:1], in0=LO[:, :, 0:1],
                            scalar1=nWt[:P], scalar2=None, op0=OP.mult)
    nc.vector.tensor_scalar(out=PG[:, :, 1:2], in0=LO[:, :, 1:2],
                            scalar1=nHt[:P], scalar2=None, op0=OP.mult)
    nc.vector.tensor_scalar(out=PG[:, :, 2:3], in0=HI[:, :, 0:1],
                            scalar1=Wt[:P], scalar2=None, op0=OP.mult)
    nc.vector.tensor_scalar(out=PG[:, :, 3:4], in0=HI[:, :, 1:2],
                            scalar1=Ht[:P], scalar2=None, op0=OP.mult)
    WD = col.tile([P, NCH, 2], F32, tag="WD")
    nc.vector.tensor_tensor(out=WD, in0=HI, in1=LO, op=OP.subtract)
    nc.vector.tensor_scalar(out=PG[:, :, 4:5], in0=WD[:, :, 0:1],
                            scalar1=Wt[:P], scalar2=None, op0=OP.mult)
    nc.vector.tensor_scalar(out=PG[:, :, 5:6], in0=WD[:, :, 1:2],
                            scalar1=Ht[:P], scalar2=None, op0=OP.mult)
    nc.vector.tensor_tensor(out=PG[:, :, 6:7], in0=PG[:, :, 4:5],
                            in1=PG[:, :, 5:6], op=OP.mult)
    C2 = col.tile([P, NCH, 2], F32, tag="C2")
    nc.vector.tensor_tensor(out=C2, in0=LO, in1=HI, op=OP.add)
    nc.vector.tensor_scalar(out=PG[:, :, 7:8], in0=C2[:, :, 0:1],
                            scalar1=nWt[:P], scalar2=0.5, op0=OP.mult,
                            op1=OP.mult)
    nc.vector.tensor_scalar(out=PG[:, :, 8:9], in0=C2[:, :, 1:2],
                            scalar1=nHt[:P], scalar2=0.5, op0=OP.mult,
                            op1=OP.mult)

    # diag(score) per chunk, bf16
    DG = scal.tile([P, NCH, P], F32, tag="DG")
    for j in range(NCH):
        nc.vector.tensor_scalar(out=DG[:, j, :], in0=identf[:P, :P],
                                scalar1=st["SCOREc"][:, j:j + 1], scalar2=LAM,
                                op0=OP.mult, op1=OP.mult)

    # --- rel rows: 9 rows per map (x0,y0,x1,y1,w,h,area,vx,vy), pixels
    RVt = io.tile([P, NCH, 4], F32, tag="RVt")
    nc.sync.dma_start(out=RVt, in_=rv[b].rearrange("(j p) c -> p j c", p=P))
    RB9f = rows.tile([6, 2 * NR], F32, tag="RB9f")
    RB9b = rows.tile([3, 2 * NR], FP16, tag="RB9b")
    for m, (boxd, vc) in enumerate(((rsb, 0), (rob, 2))):
        LO, HI = box_lohi(boxd)
        RX = col.tile([P, NCH, 9], F32, tag="RX")
        nc.vector.tensor_scalar(out=RX[:, :, 0:1], in0=LO[:, :, 0:1],
                                scalar1=Wt[:P], scalar2=None, op0=OP.mult)
        nc.vector.tensor_scalar(out=RX[:, :, 1:2], in0=LO[:, :, 1:2],
                                scalar1=Ht[:P], scalar2=None, op0=OP.mult)
        nc.vector.tensor_scalar(out=RX[:, :, 2:3], in0=HI[:, :, 0:1],
                                scalar1=Wt[:P], scalar2=None, op0=OP.mult)
        nc.vector.tensor_scalar(out=RX[:, :, 3:4], in0=HI[:, :, 1:2],
                                scalar1=Ht[:P], scalar2=None, op0=OP.mult)
        WD2 = col.tile([P, NCH, 2], F32, tag="WD2")
        nc.vector.tensor_tensor(out=WD2, in0=HI, in1=LO, op=OP.subtract)
        nc.vector.tensor_scalar(out=RX[:, :, 4:5], in0=RVt[:, :, vc:vc + 1],
                                scalar1=Wt[:P], scalar2=None, op0=OP.mult)
        nc.vector.tensor_scalar(out=RX[:, :, 5:6],
                                in0=RVt[:, :, vc + 1:vc + 2],
                                scalar1=Ht[:P], scalar2=None, op0=OP.mult)
        nc.vector.tensor_scalar(out=RX[:, :, 6:7], in0=WD2[:, :, 0:1],
                                scalar1=Wt[:P], scalar2=None, op0=OP.mult)
        nc.vector.tensor_scalar(out=RX[:, :, 7:8], in0=WD2[:, :, 1:2],
                                scalar1=Ht[:P], scalar2=None, op0=OP.mult)
        nc.vector.tensor_tensor(out=RX[:, :, 8:9], in0=RX[:, :, 6:7],
                                in1=RX[:, :, 7:8], op=OP.mult)
        for j in range(NCH):
            PSU = ps.tile([128, 512], F32, tag="PSU")
            nc.tensor.transpose(PSU[:6, 0:P], RX[:, j, 0:6], identf[:P, :P])
            nc.tensor.transpose(PSU[:3, 200:200 + P], RX[:, j, 6:9],
                                identf[:P, :P])
            csl = slice(m * NR + P * j, m * NR + P * (j + 1))
            nc.scalar.copy(out=RB9f[0:6, csl], in_=PSU[0:6, 0:P])
            nc.scalar.copy(out=RB9b[0:3, csl], in_=PSU[0:3, 200:200 + P])
    RDf = dr.tile([6, 2 * NR], F32, tag="RDf", name=f"RDf{b}")
    nc.sync.dma_start(out=RDf, in_=RB9f)
    RDb = dr.tile([3, 2 * NR], FP16, tag="RDb", name=f"RDb{b}")
    nc.sync.dma_start(out=RDb, in_=RB9b)
    R = {}
    for k, nm_ in enumerate(("RX0", "RY0", "RX1", "RY1", "RVX", "RVY")):
        R[nm_] = rows.tile([128, 2 * NR], F32, tag=f"ROWF{k}",
                           name=f"ROWF{k}_{b}")
        nc.sync.dma_start(out=R[nm_], in_=_bcast(RDf[k:k + 1, :], 128))
    for k, nm_ in enumerate(("RW", "RH", "RAREA")):
        R[nm_] = rows.tile([128, 2 * NR], FP16, tag=f"ROWB{k}",
                           name=f"ROWB{k}_{b}")
        nc.sync.dma_start(out=R[nm_], in_=_bcast(RDb[k:k + 1, :], 128))
    st["PG"] = PG
    st["DG"] = DG
    st["R"] = R


def _emit_out(env, b, st, item):
    nc = env["nc"]
    mp, pso = env["mp"], env["pso"]
    out_s, out_o = env["out_s"], env["out_o"]
    DG = st["DG"]
    nm, j = item
    sl = slice(P * j, P * (j + 1))
    for m_i, odram in ((0, out_s), (1, out_o)):
        OUTT = pso.tile([128, 512], F32, tag="OUTT",
                        name=f"OUTT{m_i}_{b}_{j}")
        for k in range(NCH):
            nc.tensor.matmul(OUTT[:P, P * k:P * (k + 1)],
                             lhsT=nm[:, m_i * NR + P * k:m_i * NR + P * (k + 1)],
                             rhs=DG[:, j, :], start=True, stop=True)
        OUTS = mp.tile([P, NR], F32, tag="OUTS",
                       name=f"OUTS{m_i}_{b}_{j}")
        nc.scalar.copy(out=OUTS, in_=OUTT[:P, 0:NR])
        nc.sync.dma_start(
            out=odram[b, :, sl].rearrange("(k p) e -> p k e", p=P),
            in_=OUTS.rearrange("p (k e) -> p k e", k=NCH))


def _maps(env, b, st):
    nc = env["nc"]
    mp, ps, pso = env["mp"], env["ps"], env["pso"]
    mph = env["mph"]
    HEAD = {"s", "rxA", "rxB", "ryA", "ryB", "dxn", "dyn", "i1", "mu"}
    out_s, out_o = env["out_s"], env["out_o"]
    PAe, PBe, PAr, PBr = st["PAe"], st["PBe"], st["PAr"], st["PBr"]
    PG, DG, R, ESQc = st["PG"], st["DG"], st["R"], st["ESQc"]
    RX0, RY0, RX1, RY1, RVX, RVY, RW, RH, RAREA = (
        R[k] for k in ("RX0", "RY0", "RX1", "RY1", "RVX", "RVY",
                       "RW", "RH", "RAREA"))

    pend = []
    for j in range(NCH):
        sl = slice(P * j, P * (j + 1))
        pg = lambda c: PG[:, j, c:c + 1]

        D2s = ps.tile([128, 512], F32, tag="PSU", name=f"D2s_{b}_{j}")
        nc.tensor.matmul(D2s[:P, 0:NR], lhsT=PAe[:, sl], rhs=PAr[:, 0:NR],
                         start=True, stop=False)
        nc.tensor.matmul(D2s[:P, 0:NR], lhsT=PBe[:, sl], rhs=PBr[:, 0:NR],
                         start=False, stop=True)
        D2o = ps.tile([128, 512], F32, tag="PSU", name=f"D2o_{b}_{j}")
        nc.tensor.matmul(D2o[:P, 0:NR], lhsT=PAe[:, sl],
                         rhs=PAr[:, NR:2 * NR], start=True, stop=False)
        nc.tensor.matmul(D2o[:P, 0:NR], lhsT=PBe[:, sl],
                         rhs=PBr[:, NR:2 * NR], start=False, stop=True)

        def m(tag, dt=FP16):
            pool = mph if tag in HEAD else mp
            return pool.tile([P, 2 * NR], dt, tag=tag, name=f"{tag}_{b}_{j}")

        # ACT: abs legs of the L1 rel-vec distance (row-dependent: no wait)
        ax = m("ax")
        nc.scalar.activation(out=ax, in_=RVX[:P], func=AF.Abs, bias=pg(7))
        ay = m("ay")
        nc.scalar.activation(out=ay, in_=RVY[:P], func=AF.Abs, bias=pg(8))
        # DVE: SA = rarea + earea
        SA = m("SA")
        nc.vector.tensor_scalar(out=SA, in0=RAREA[:P], scalar1=pg(6),
                                scalar2=None, op0=OP.add)

        # Pool: asum = ax + ay
        eng = nc.gpsimd if USE_POOL else nc.vector
        asum = ax if USE_INPLACE else m("asum")
        eng.tensor_tensor(out=asum, in0=ax, in1=ay, op=OP.add)

        # ACT: relu legs of intersection bounds (exact f32 differences)
        rxA = m("rxA", F32)
        nc.scalar.activation(out=rxA, in_=RX0[:P], func=AF.Relu, bias=pg(0))
        rxB = m("rxB", F32)
        nc.scalar.activation(out=rxB, in_=RX1[:P], func=AF.Relu, bias=pg(2),
                             scale=-1.0)
        ryA = m("ryA", F32)
        nc.scalar.activation(out=ryA, in_=RY0[:P], func=AF.Relu, bias=pg(1))
        ryB = m("ryB", F32)
        nc.scalar.activation(out=ryB, in_=RY1[:P], func=AF.Relu, bias=pg(3),
                             scale=-1.0)
        # ACT: s = sqrt(D2 + esq); s1 = s + 1 (matmul-dependent)
        s = m("s")
        nc.scalar.activation(out=s[:, 0:NR], in_=D2s[:P, 0:NR], func=AF.Sqrt,
                             bias=ESQc[:, j:j + 1])
        nc.scalar.activation(out=s[:, NR:2 * NR], in_=D2o[:P, 0:NR],
                             func=AF.Sqrt, bias=ESQc[:, j:j + 1])
        if USE_INPLACE:
            nc.scalar.activation(out=s, in_=s, func=AF.Copy, bias=1.0)
            s1 = s
        else:
            s1 = m("s1")
            nc.scalar.activation(out=s1, in_=s, func=AF.Copy, bias=1.0)
        # DVE: dxn = -dx = rxA + rxB - ew (f32 in, small bf16 out)
        dxn = m("dxn")
        nc.vector.scalar_tensor_tensor(out=dxn, in0=rxA, scalar=pg(4),
                                       in1=rxB, op0=OP.subtract, op1=OP.add)
        dyn = m("dyn")
        nc.vector.scalar_tensor_tensor(out=dyn, in0=ryA, scalar=pg(5),
                                       in1=ryB, op0=OP.subtract, op1=OP.add)
        i1 = m("i1")
        nc.vector.scalar_tensor_tensor(out=i1, in0=dxn, scalar=0.0, in1=dyn,
                                       op0=OP.min, op1=OP.mult)
        mu = m("mu")
        nc.vector.scalar_tensor_tensor(out=mu, in0=i1, scalar=0.0, in1=SA,
                                       op0=OP.max, op1=OP.subtract)
        u2 = m("u2")
        nc.vector.tensor_tensor(out=u2, in0=mu, in1=mu, op=OP.mult)
        ca = i1 if USE_INPLACE else m("ca")
        nc.vector.scalar_tensor_tensor(out=ca, in0=i1, scalar=0.0, in1=mu,
                                       op0=OP.max, op1=OP.add)
        wc = m("wc")
        nc.vector.scalar_tensor_tensor(out=wc, in0=RW[:P], scalar=pg(4),
                                       in1=dxn, op0=OP.add, op1=OP.add)
        hc = m("hc")
        nc.vector.scalar_tensor_tensor(out=hc, in0=RH[:P], scalar=pg(5),
                                       in1=dyn, op0=OP.add, op1=OP.add)
        areac = wc if USE_INPLACE else m("areac")
        eng.tensor_tensor(out=areac, in0=wc, in1=hc, op=OP.mult)
        m1 = hc if USE_INPLACE else m("m1")
        eng.tensor_tensor(out=m1, in0=areac, in1=ca, op=OP.mult)
        Nn = m1 if USE_INPLACE else m("Nn")
        eng.tensor_tensor(out=Nn, in0=m1, in1=u2, op=OP.add)
        P1 = mu if USE_INPLACE else m("P1")
        nc.vector.scalar_tensor_tensor(out=P1, in0=mu, scalar=-1.0, in1=areac,
                                       op0=OP.mult, op1=OP.mult)
        den = asum if USE_INPLACE else m("den")
        nc.vector.scalar_tensor_tensor(out=den, in0=asum, scalar=LAM, in1=s1,
                                       op0=OP.add, op1=OP.mult)
        D3 = m("D3", F32)
        eng.tensor_tensor(out=D3, in0=P1, in1=den, op=OP.mult)
        r3 = rxB if USE_INPLACE else m("r3", F32)
        nc.vector.reciprocal(out=r3, in_=D3)
        nm = ryB if USE_INPLACE else m("nm", F32)
        nc.vector.scalar_tensor_tensor(out=nm, in0=Nn, scalar=0.0, in1=r3,
                                       op0=OP.max, op1=OP.mult)

        dtag = os.environ.get('K2_DUMP')
        if dtag:
            srcs = {"s": s, "s1": s1, "SA": SA, "ax": ax, "ay": ay,
                    "den": den, "i1": i1, "mu": mu, "u2": u2, "ca": ca,
                    "wc": wc, "hc": hc, "areac": areac, "m1": m1, "Nn": Nn,
                    "P1": P1, "D3": D3, "r3": r3, "nm": nm,
                    "dxn": dxn, "dyn": dyn}
            DBF = mp.tile([P, 2 * NR], F32, tag="DBF", name=f"DBF_{b}_{j}")
            nc.scalar.copy(out=DBF, in_=srcs[dtag])
            nc.sync.dma_start(out=env["dbg"][b, j], in_=DBF)

        pend.append((nm, j))
        if len(pend) > 1:
            _emit_out(env, b, st, pend.pop(0))
    while pend:
        _emit_out(env, b, st, pend.pop(0))


# ---------------------------------------------------------------------------
# SPMD compile/run wrapper
# ---------------------------------------------------------------------------


class _CompiledKernel:
    """Compiled SPMD executable: jit built once, reusable across calls."""

    def __init__(self, nc, n_cores):
        import jax
        from jax.sharding import Mesh, PartitionSpec
        try:
            from jax.experimental.shard_map import shard_map
        except Exception:
            from jax.shard_map import shard_map
        from concourse import bass2jax
        from concourse.bass2jax import _bass_exec_p, install_neuronx_cc_hook

        install_neuronx_cc_hook()
        self.jax = jax
        self.nc = nc
        self.n_cores = n_cores
        partition_name = (nc.partition_id_tensor.name
                          if nc.partition_id_tensor else None)
        self.partition_name = partition_name
        in_names, out_names, out_avals, zero_outs = [], [], [], []
        for alloc in nc.m.functions[0].allocations:
            if not isinstance(alloc, mybir.MemoryLocationSet):
                continue
            name = alloc.memorylocations[0].name
            if alloc.kind == "ExternalInput":
                if name != partition_name:
                    in_names.append(name)
            elif alloc.kind == "ExternalOutput":
                shape = tuple(alloc.tensor_shape)
                dtype = mybir.dt.np(alloc.dtype)
                out_names.append(name)
                out_avals.append(jax.core.ShapedArray(shape, dtype))
                zero_outs.append(np.zeros(shape, dtype))
        self.in_names = in_names
        self.out_names = out_names
        self.out_avals = out_avals
        self.zero_outs = zero_outs
        all_in = in_names + out_names
        if partition_name is not None:
            all_in.append(partition_name)

        def _exec(ins, outs):
            operands = list(ins) + list(outs)
            if partition_name is not None:
                operands.append(bass2jax.partition_id_tensor())
            return tuple(_bass_exec_p.bind(
                *operands,
                out_avals=tuple(out_avals),
                in_names=tuple(all_in),
                out_names=tuple(out_names),
                lowering_input_output_aliases=(),
                sim_require_finite=True,
                sim_require_nnan=True,
                nc=nc,
            ))

        self._exec = _exec

        def _body(*args):
            return _exec(args[:len(in_names)], args[len(in_names):])

        devices = jax.devices()[:n_cores]
        self._mesh = Mesh(np.asarray(devices), ("core",))
        self._shard_map = shard_map
        self._pspec = PartitionSpec
        nin = len(in_names) + len(out_names)
        self._fn = jax.jit(
            shard_map(_body, mesh=self._mesh,
                      in_specs=(PartitionSpec("core"),) * nin,
                      out_specs=(PartitionSpec("core"),) * len(out_names),
                      check_rep=False),
            keep_unused=True)

    def chained_fn(self, reps):
        """jit fn running `reps` kernel executions serially on-device."""
        jax = self.jax
        n_in = len(self.in_names)

        def _body(*args):
            ins = args[:n_in]
            outs = tuple(args[n_in:])

            def step(_, carry):
                return self._exec(ins, carry)

            outs = jax.lax.fori_loop(0, reps, step, outs)
            return outs

        return jax.jit(
            self._shard_map(_body, mesh=self._mesh,
                            in_specs=(self._pspec("core"),) * (n_in + len(self.out_names)),
                            out_specs=(self._pspec("core"),) * len(self.out_names),
                            check_rep=False),
            keep_unused=True)

    def run(self, in_maps):
        jax = self.jax
        n = self.n_cores
        per_core = [[np.asarray(m[nm]) for nm in self.in_names]
                    for m in in_maps]
        concat_in = [np.concatenate([per_core[c][i] for c in range(n)], axis=0)
                     for i in range(len(self.in_names))]
        concat_zero = [np.zeros((n * z.shape[0], *z.shape[1:]), z.dtype)
                       for z in self.zero_outs]
        outs = jax.block_until_ready(self._fn(*concat_in, *concat_zero))
        return [
            {nm: np.asarray(outs[i]).reshape(n, *self.out_avals[i].shape)[c]
             for i, nm in enumerate(self.out_names)}
            for c in range(n)
        ]


_CACHE = {}


def _get_nc():
    if "nc" not in _CACHE:
        _CACHE["nc"] = _build(N_IMG)
    return _CACHE["nc"]


def _get_ck():
    if "ck" not in _CACHE:
        _CACHE["ck"] = _CompiledKernel(_get_nc(), N_CORES)
    return _CACHE["ck"]


def kernel(pred_boxes, pred_logits, pred_rel_obj_logits, pred_rel_sub_logits,
           pred_rel_obj_box, pred_rel_sub_box, pred_rel_vec, target_sizes):
    inp = {
        "pred_boxes": np.ascontiguousarray(pred_boxes, dtype=np.float32),
        "pred_logits": np.ascontiguousarray(pred_logits, dtype=np.float32),
        "pred_rel_obj_logits": np.ascontiguousarray(pred_rel_obj_logits, dtype=np.float32),
        "pred_rel_sub_logits": np.ascontiguousarray(pred_rel_sub_logits, dtype=np.float32),
        "pred_rel_obj_box": np.ascontiguousarray(pred_rel_obj_box, dtype=np.float32),
        "pred_rel_sub_box": np.ascontiguousarray(pred_rel_sub_box, dtype=np.float32),
        "pred_rel_vec": np.ascontiguousarray(pred_rel_vec, dtype=np.float32),
        "target_sizes": np.ascontiguousarray(target_sizes, dtype=np.float32),
    }
    in_maps = [{k: v[c * N_IMG:(c + 1) * N_IMG] for k, v in inp.items()}
               for c in range(N_CORES)]
    res = None
    try:
        res = _get_ck().run(in_maps)
    except Exception:
        import time as _time
        _time.sleep(2.0)
        try:
            res = _get_ck().run(in_maps)
        except Exception:
            r = bass_utils.run_bass_kernel_spmd(
                _get_nc(), in_maps, core_ids=list(range(N_CORES)))
            res = r.results
    sub = np.concatenate([res[c]["out_sub"] for c in range(N_CORES)], axis=0)
    obj = np.concatenate([res[c]["out_obj"] for c in range(N_CORES)], axis=0)
    return sub, obj
